# revision 15
# baseline (speedup 1.0000x reference)
"""Trainium2 kernel for nn_Lnlv_71519795413647.

Video-moment-localization model: bidirectional GRU encoders, cross-modal
additive attention, two GRU interactors, causal self-attention, scoring
head.

Device strategy (per spec sharding hint): the (T,T,H) self-attention
tanh score tensor is sharded across the 8 NeuronCores over the T query
axis (sequence parallel).  Two extra levers vs. the naive version:

 * causality: the row-i softmax is masked to j >= i, so only the upper
   triangle of (T,T) is computed — half the tanh work.  Rows are dealt
   cyclically (core c owns rows i = c + 8k) so every core sees the same
   per-row lengths FD_k = 512 - 8k: the SPMD program is identical on all
   cores and the load is balanced to <3%.
 * engine balance: the q_i + k_j pre-add runs on the Vector engine as a
   bf16 tensor-scalar (4x mode); the Scalar engine then does one big
   fused tanh per 8-row group (amortizing the per-instruction bubble);
   the PE reduces over the hidden axis with v as the 1-column stationary
   operand into per-row PSUM banks; the Vector engine evacuates rows
   into a flat partition-0 tile that DMAs out per group.

The strictly sequential GRU recurrences and the small remaining glue run
on host (numpy), as in the reference decomposition.

Shapes are hardcoded: T=512, S=32, VFD=1024, HID=512, HH=256, WED=300.
"""

import numpy as np

T = 512
S = 32
VFD = 1024
HID = 512
HH = HID // 2
WINDOW_SIZE = 16
N_CORES = 8
ROWS = T // N_CORES        # query rows per core (cyclic: i = c + 8k)
G = 8                      # rows per group (one fused tanh per group)
N_GROUPS = ROWS // G

_DEVICE = {"built": None}
_LAST_EXEC_NS = None
_LAST_TRACE = None


def _fit_tanh_sines(L, R):
    """Fit tanh(x) ~ sum_r w_r sin(om_r x) on [-L, L] (numpy lstsq).

    Returns (om[R], w[R], max_abs_err).  tanh and sin are both odd, so a
    fit on [0, L] covers the symmetric interval.
    """
    grid = np.linspace(0.0, L, 3001)
    om = (np.arange(1, R + 1) * np.pi / (2.0 * L)) * 0.9
    A = np.sin(np.outer(grid, om))
    w, *_ = np.linalg.lstsq(A, np.tanh(grid), rcond=None)
    err = np.abs(A @ w - np.tanh(grid)).max()
    return om.astype(np.float32), w.astype(np.float32), float(err)


# tuning knobs (sim-explored)
CFG = {
    "R": 5,               # sine terms in the separable tanh expansion
}
HALF_PI = 1.5707963267948966


def _build_s2_kernel():
    """Separable self-attention scores via a sine expansion of tanh.

    s2[i,j] = sum_h v_h tanh(q_ih + k_jh)
            ~ sum_r w_r sum_h v_h [sin(om_r q_ih) cos(om_r k_jh)
                                   + cos(om_r q_ih) sin(om_r k_jh)]

    Each r-term is a pair of rank-H matmuls: ACT computes the four trig
    matrices (om_r arrives at runtime through the activation's
    per-partition scale operand), DVE folds w_r * v into the Q-side
    factors, and the PE accumulates all 8R partial matmuls into a single
    [64, 512] PSUM bank.  Rows are sharded contiguously across cores.
    """
    import concourse.bacc as bacc
    import concourse.mybir as mybir
    import concourse.tile as tile

    R = CFG["R"]
    nc = bacc.Bacc(trn_type="TRN2", num_devices=N_CORES, debug=False)
    # K^T packed [128 part][hb][j], fp32, same on every core
    ktf = nc.dram_tensor("ktf", [128, 4, T], mybir.dt.float32, kind="ExternalInput")
    # per-core Q^T columns for the core's 64 contiguous rows, fp32
    qtf = nc.dram_tensor("qtf", [128, 4, ROWS], mybir.dt.float32, kind="ExternalInput")
    # vw[p, hb, r] = v[hb*128+p] * w_r
    vwf = nc.dram_tensor("vwf", [128, 4, R], mybir.dt.float32, kind="ExternalInput")
    # om[p, r] = om_r (replicated down partitions); last column = pi/2
    omf = nc.dram_tensor("omf", [128, R + 1], mybir.dt.float32, kind="ExternalInput")
    s2d = nc.dram_tensor("s2d", [ROWS, T], mybir.dt.float32, kind="ExternalOutput")

    Sin = mybir.ActivationFunctionType.Sin

    with tile.TileContext(nc) as tc:
        with (
            tc.tile_pool(name="const", bufs=1) as cpool,
            tc.tile_pool(name="ktrig", bufs=2) as kpool,
            tc.tile_pool(name="qtrig", bufs=4) as qpool,
            tc.tile_pool(name="lhs", bufs=4) as lpool,
            tc.tile_pool(name="out", bufs=1) as opool,
            tc.tile_pool(name="ps", bufs=1, space="PSUM") as pspool,
        ):
            kt = cpool.tile([128, 4, T], mybir.dt.float32, name="kt")
            qt = cpool.tile([128, 4, ROWS], mybir.dt.float32, name="qt")
            vw = cpool.tile([128, 4, R], mybir.dt.float32, name="vw")
            om = cpool.tile([128, R + 1], mybir.dt.float32, name="om")
            nc.sync.dma_start(kt[:], ktf[:])
            nc.sync.dma_start(qt[:], qtf[:])
            nc.sync.dma_start(vw[:], vwf[:])
            nc.sync.dma_start(om[:], omf[:])

            out_ps = pspool.tile([ROWS, T], mybir.dt.float32, name="out_ps")

            n_mm = 8 * R
            mm = 0
            for r in range(R):
                # trig factors for this term; cos(x) = sin(x + pi/2)
                ks = kpool.tile([128, 4, T], mybir.dt.float32r,
                                name=f"ks{r}", tag="ks")
                kc = kpool.tile([128, 4, T], mybir.dt.float32r,
                                name=f"kc{r}", tag="kc")
                qs = qpool.tile([128, 4, ROWS], mybir.dt.float32r,
                                name=f"qs{r}", tag="qs")
                qc = qpool.tile([128, 4, ROWS], mybir.dt.float32r,
                                name=f"qc{r}", tag="qc")
                sc = om[:, r:r + 1]
                hp = om[:, R:R + 1]
                nc.scalar.activation(qs[:], qt[:], Sin, scale=sc)
                nc.scalar.activation(qc[:], qt[:], Sin, scale=sc, bias=hp)
                nc.scalar.activation(ks[:], kt[:], Sin, scale=sc)
                nc.scalar.activation(kc[:], kt[:], Sin, scale=sc, bias=hp)

                for qtile, ktile, nm in ((qs, kc, "sc"), (qc, ks, "cs")):
                    for hb in range(4):
                        lhs = lpool.tile([128, ROWS], mybir.dt.float32r,
                                         name=f"l{nm}{r}_{hb}", tag="lhs")
                        nc.vector.tensor_scalar_mul(
                            lhs[:], qtile[:, hb, :], vw[:, hb, r:r + 1])
                        nc.tensor.matmul(
                            out_ps[:],
                            lhs[:],
                            ktile[:, hb, :],
                            start=(mm == 0),
                            stop=(mm == n_mm - 1),
                        )
                        mm += 1

            s2sb = opool.tile([ROWS, T], mybir.dt.float32, name="s2sb")
            nc.vector.tensor_copy(s2sb[:], out_ps[:])
            nc.sync.dma_start(s2d[:], s2sb[:])
    nc.compile()
    return nc


def _device_s2(Qr, Kr, v):
    """s2[i, j] = v . tanh(Qr[i] + Kr[j]) on 8 cores, rows sharded."""
    from concourse import bass_utils

    R = CFG["R"]
    # runtime sine fit of tanh over the actual q+k range
    L_data = (np.abs(Qr).max(axis=0) + np.abs(Kr).max(axis=0)).max()
    L = max(float(L_data) * 1.02, 0.35)
    om, w, fit_err = _fit_tanh_sines(L, R)
    if fit_err > 2e-3 or not np.isfinite(fit_err):
        # pathological input range: exact (slow) host path
        return np.tanh(Qr[:, None, :] + Kr[None, :, :]) @ v

    # K^T packed [p][hb][j]: kt[p, hb, j] = Kr[j, hb*128 + p]
    ktc = np.ascontiguousarray(
        Kr.T.reshape(4, 128, T).transpose(1, 0, 2)).astype(np.float32)
    # vw[p, hb, r] = v[hb*128 + p] * w_r
    vwc = np.ascontiguousarray(
        v.reshape(4, 128).T[:, :, None] * w[None, None, :]).astype(np.float32)
    om_ext = np.concatenate([om, np.float32([HALF_PI])])
    omc = np.ascontiguousarray(np.broadcast_to(om_ext, (128, R + 1))).astype(np.float32)

    if _DEVICE["built"] is None:
        _DEVICE["built"] = _build_s2_kernel()
    nc = _DEVICE["built"]

    in_maps = []
    for c in range(N_CORES):
        q_rows = Qr[c * ROWS:(c + 1) * ROWS]         # (ROWS, HID)
        qtc = np.ascontiguousarray(
            q_rows.T.reshape(4, 128, ROWS).transpose(1, 0, 2)).astype(np.float32)
        in_maps.append({"ktf": ktc, "qtf": qtc, "vwf": vwc, "omf": omc})
    try:
        res = bass_utils.run_bass_kernel_spmd(nc, in_maps, list(range(N_CORES)))
    except ModuleNotFoundError:
        # BASS_TRACE was requested but the axon NTFF hook isn't importable
        # in this environment; rerun untraced rather than failing.
        import os
        os.environ["BASS_NEVER_TRACE"] = "1"
        try:
            res = bass_utils.run_bass_kernel_spmd(nc, in_maps, list(range(N_CORES)))
        finally:
            os.environ.pop("BASS_NEVER_TRACE", None)
    import sys
    mod = sys.modules[__name__]
    mod._LAST_EXEC_NS = res.exec_time_ns
    mod._LAST_TRACE = res.instructions_and_trace[1] if res.instructions_and_trace else None
    return np.concatenate([res.results[c]["s2d"] for c in range(N_CORES)], axis=0)


# ---------------------------------------------------------------------------
# host-side model math
# ---------------------------------------------------------------------------


def _sigmoid(x):
    return 1.0 / (1.0 + np.exp(-x))


def _gru_seq(x, Wih, Whh, bih, bhh):
    Tn = x.shape[0]
    H = Whh.shape[0]
    pre = x @ Wih + bih  # (T, 3H)
    h = np.zeros((H,), np.float32)
    ys = np.empty((Tn, H), np.float32)
    for t in range(Tn):
        ph = h @ Whh + bhh
        pi = pre[t]
        r = _sigmoid(pi[:H] + ph[:H])
        z = _sigmoid(pi[H : 2 * H] + ph[H : 2 * H])
        n = np.tanh(pi[2 * H :] + r * ph[2 * H :])
        h = (1.0 - z) * n + z * h
        ys[t] = h
    return ys


def _bigru(x, Wih, Whh, bih, bhh):
    f = _gru_seq(x, Wih[0], Whh[0], bih[0], bhh[0])
    b = _gru_seq(x[::-1], Wih[1], Whh[1], bih[1], bhh[1])[::-1]
    return np.concatenate([f, b], axis=-1)


def _softmax(x, axis):
    m = np.max(x, axis=axis, keepdims=True)
    e = np.exp(x - m)
    return e / np.sum(e, axis=axis, keepdims=True)


def kernel(video, text, vp_W, vp_b, vgru_Wih, vgru_Whh, vgru_bih, vgru_bhh,
           emb, tp_W, tp_b, tgru_Wih, tgru_Whh, tgru_bih, tgru_bhh,
           cma_Wq, cma_bq, cma_Wk, cma_bk, cma_v,
           cm_gru_Wih, cm_gru_Whh, cm_gru_bih, cm_gru_bhh,
           si_Wq, si_bq, si_Wk, si_bk, si_v,
           si_gru_Wih, si_gru_Whh, si_gru_bih, si_gru_bhh,
           wp_W1, wp_b1, wp_v, cp_W1, cp_b1, cp_v):
    f32 = lambda a: np.asarray(a, np.float32)
    video = f32(video)
    text = np.asarray(text)

    # encoders
    H_v = _bigru(video @ f32(vp_W) + f32(vp_b), f32(vgru_Wih), f32(vgru_Whh),
                 f32(vgru_bih), f32(vgru_bhh))  # (T, HID)
    H_s = _bigru(f32(emb)[text] @ f32(tp_W) + f32(tp_b), f32(tgru_Wih),
                 f32(tgru_Whh), f32(tgru_bih), f32(tgru_bhh))  # (S, HID)

    # cross-modal additive attention
    Qv = H_v @ f32(cma_Wq) + f32(cma_bq)  # (T, HID)
    Ks = H_s @ f32(cma_Wk) + f32(cma_bk)  # (S, HID)
    e = np.tanh(Qv[:, None, :] + Ks[None, :, :])  # (T,S,HID)
    w = _softmax(e @ f32(cma_v), axis=1)  # (T,S)
    h_s_bar = w @ H_s  # (T, HID)

    h_v_t = np.maximum(H_v, 0.0) * h_s_bar
    h_s_t = np.maximum(h_s_bar, 0.0) * h_v_t
    h_r = _gru_seq(np.concatenate([h_v_t, h_s_t], axis=1).astype(np.float32),
                   f32(cm_gru_Wih), f32(cm_gru_Whh), f32(cm_gru_bih),
                   f32(cm_gru_bhh))  # (T, HID)

    # self interactor: sharded on the 8 NeuronCores
    Qr = h_r @ f32(si_Wq) + f32(si_bq)
    Kr = h_r @ f32(si_Wk) + f32(si_bk)
    s2 = _device_s2(Qr, Kr, f32(si_v))
    mask = np.arange(T)[None, :] >= np.arange(T)[:, None]
    s2 = np.where(mask, s2, np.float32(-1e30))
    att = _softmax(s2, axis=1) @ h_r  # (T, HID)
    h_d = _gru_seq(np.concatenate([h_r, att], axis=1).astype(np.float32),
                   f32(si_gru_Wih), f32(si_gru_Whh), f32(si_gru_bih),
                   f32(si_gru_bhh))  # (T, HID)

    # segment localizer (softmax over axis of size 1 -> ones)
    h_o = np.sum(H_s, axis=0)  # (HID,)
    cat = np.concatenate([h_d, np.broadcast_to(h_o, h_d.shape)], axis=1)
    frame_scores = np.tanh(cat @ f32(cp_W1) + f32(cp_b1)) @ f32(cp_v)  # (T,)

    n_win = T - WINDOW_SIZE + 1
    window_scores = frame_scores[:n_win].astype(np.float32)
    window_starts = np.arange(n_win, dtype=np.int32)
    return (window_scores, window_starts)
